# revision 7
# baseline (speedup 1.0000x reference)
"""Trainium2 Bass kernel for nn_GateCircuit (14-qubit batched gate circuit).

Math: the reference applies RX(x@W.T[:,i]) then RY(params[i]) on wire i of
|0...0> (a product state stays a product state since each gate hits a distinct
wire), then a CNOT ladder CNOT(i, i+1), then measures <Z_0>.  Qubit 0 is only
ever a CNOT *control*, so its marginal is untouched by the ladder; the
expectation collapses to the single-qubit value

    <Z_0> = cos(x @ W[0]) * cos(params[0])
    out   = sigmoid(<Z_0>)

Sharding: pure data parallel, batch 4096 split 512 per core across 8 cores.

Host prep (scalar/row transforms only; the 4MB matvec stays on-device):
  w' = W[0] / (2*pi), and a per-call least-squares fit of
  F(v) = sigmoid(c0*cos(2*pi*sqrt(v))) on v in [0, 0.25] as a degree-4
  polynomial (c0 = cos(params[0]) known at call time; worst-case fit error
  1.8e-3 abs vs the 2e-2 rel tolerance).  w' + 5 coefficients form one
  [1, 261] row, replicated to 128 partitions by a stride-0 broadcast DMA
  (1KB HBM read instead of 132KB).

On-device per core (all f32, DVE only -- no ACT tables, no table loads):
  z' = x @ w'             4x DVE scalar_tensor_tensor with accumulator
                          (z' = z/2pi; x laid out [128, 4, 256]: partition p
                          holds rows 4p..4p+3; quarter-DMAs spread over the
                          Scalar/Sync hardware-DGE queues + the GpSimd
                          software queue, dots ordered by landing time)
  k  = (z' + M) - M       M = 1.5*2^23: fp32 round-to-nearest
  v  = (k - z')^2         = y^2, y = frac distance, |y| <= 0.5 (sign-free)
  out= F(v)               Horner via TS/STT alternation, coeffs as
                          per-partition [128,1] APs
"""

import math

import numpy as np

_NCORES = 8
_B = 4096
_F = 256
_BS = _B // _NCORES  # 512 samples per core
_NT = _BS // 128     # 4 samples per partition
_INV_TWO_PI = float(1.0 / (2.0 * math.pi))
_MAGIC = float(1.5 * 2 ** 23)  # fp32 round-to-nearest-integer bias
_DEG = 4             # F(v) polynomial degree
_NCF = _DEG + 1      # number of coefficients

_CACHE: dict = {}


def _build():
    import concourse.bacc as bacc
    import concourse.mybir as mybir
    import concourse.tile as tile

    f32 = mybir.dt.float32
    Alu = mybir.AluOpType

    nc = bacc.Bacc("TRN2", target_bir_lowering=False, debug=False,
                   num_devices=_NCORES)

    x_d = nc.dram_tensor("x", [_BS, _F], f32, kind="ExternalInput")
    w_d = nc.dram_tensor("w", [1, _F + _NCF], f32, kind="ExternalInput")
    o_d = nc.dram_tensor("o", [_BS], f32, kind="ExternalOutput")

    with tile.TileContext(nc) as tc:
        with (
            tc.tile_pool(name="xin", bufs=1) as xpool,
            tc.tile_pool(name="scratch", bufs=2) as spool,
            tc.tile_pool(name="small", bufs=1) as zpool,
        ):
            # x quarter n holds rows 4p+n (1KB contiguous per partition).
            # Queues: Scalar HW = [w broadcast, q0], Sync HW = [q1, q2],
            # GpSimd SW = [q3]; dots run in landing order q1,q0,q2,q3.
            xt = xpool.tile([128, _NT * _F], f32)
            xr = x_d.ap().rearrange("(p n) f -> p (n f)", n=_NT)
            wz = zpool.tile([128, _F + _NCF], f32)
            nc.scalar.dma_start(wz[:], w_d.ap().partition_broadcast(128))
            nc.sync.dma_start(xt[:, _F:2 * _F], xr[:, _F:2 * _F])
            nc.scalar.dma_start(xt[:, 0:_F], xr[:, 0:_F])
            nc.sync.dma_start(xt[:, 2 * _F:3 * _F], xr[:, 2 * _F:3 * _F])
            nc.gpsimd.dma_start(xt[:, 3 * _F:], xr[:, 3 * _F:])

            # z'[p, n] = sum_f x[4p+n, f] * w'[f]
            z = zpool.tile([128, _NT], f32)
            for n in (1, 0, 2, 3):
                prod = spool.tile([128, _F], f32, name=f"prod{n}")
                nc.vector.scalar_tensor_tensor(
                    prod[:], xt[:, n * _F:(n + 1) * _F], 1.0, wz[:, 0:_F],
                    op0=Alu.mult, op1=Alu.mult,
                    accum_out=z[:, n:n + 1],
                )

            # v = (round(z') - z')^2 via the fp32 magic-number trick
            a1 = zpool.tile([128, _NT], f32)
            nc.vector.tensor_scalar(a1[:], z[:], 1.0, _MAGIC,
                                    op0=Alu.mult, op1=Alu.add)
            ny = zpool.tile([128, _NT], f32)
            nc.vector.scalar_tensor_tensor(ny[:], a1[:], -_MAGIC, z[:],
                                           op0=Alu.add, op1=Alu.subtract)
            v = zpool.tile([128, _NT], f32)
            nc.vector.tensor_tensor(v[:], ny[:], ny[:], op=Alu.mult)

            # out = f0 + v*(f1 + v*(f2 + v*(f3 + v*f4)))
            def cf(k):
                return wz[:, _F + k:_F + k + 1]

            t1 = zpool.tile([128, _NT], f32)
            nc.vector.tensor_scalar(t1[:], v[:], cf(4), cf(3),
                                    op0=Alu.mult, op1=Alu.add)
            t2 = zpool.tile([128, _NT], f32)
            nc.vector.scalar_tensor_tensor(t2[:], t1[:], 0.0, v[:],
                                           op0=Alu.bypass, op1=Alu.mult)
            t3 = zpool.tile([128, _NT], f32)
            nc.vector.scalar_tensor_tensor(t3[:], t2[:], cf(2), v[:],
                                           op0=Alu.add, op1=Alu.mult)
            t4 = zpool.tile([128, _NT], f32)
            nc.vector.scalar_tensor_tensor(t4[:], t3[:], cf(1), v[:],
                                           op0=Alu.add, op1=Alu.mult)
            ot = zpool.tile([128, _NT], f32)
            nc.vector.tensor_scalar(ot[:], t4[:], 1.0, cf(0),
                                    op0=Alu.mult, op1=Alu.add)

            nc.sync.dma_start(o_d.ap().rearrange("(p n) -> p n", n=_NT), ot[:])

    nc.compile()
    return nc


def _get_nc():
    if "nc" not in _CACHE:
        _CACHE["nc"] = _build()
    return _CACHE["nc"]


def _fit_coeffs(c0: float) -> np.ndarray:
    """Least-squares fit of sigmoid(c0*cos(2*pi*sqrt(v))) on v in [0,.25],
    degree _DEG, on Chebyshev-spaced nodes (near-minimax)."""
    t = 0.5 * (1.0 - np.cos(np.pi * np.linspace(0.0, 1.0, 401))) * 0.25
    F = 1.0 / (1.0 + np.exp(-c0 * np.cos(2.0 * np.pi * np.sqrt(t))))
    A = np.stack([t ** k for k in range(_NCF)], axis=1)
    coef, *_ = np.linalg.lstsq(A, F, rcond=None)
    return coef.astype(np.float32)


def _in_maps(x, W, params):
    x = np.ascontiguousarray(np.asarray(x, dtype=np.float32))
    W = np.asarray(W, dtype=np.float32)
    params = np.asarray(params, dtype=np.float32)
    wc = np.empty((1, _F + _NCF), dtype=np.float32)
    wc[0, :_F] = W[0] * _INV_TWO_PI
    wc[0, _F:] = _fit_coeffs(math.cos(float(params[0])))
    return [
        {"x": x[c * _BS:(c + 1) * _BS], "w": wc}
        for c in range(_NCORES)
    ]


def run_spmd(x, W, params, **kw):
    """Compile (cached) and run on 8 cores; returns BassKernelResults.

    Retries a few times: the axon-relayed device occasionally reports a
    transient NRT_EXEC_UNIT_UNRECOVERABLE that clears on the next attempt.
    """
    import time

    from concourse import bass_utils

    nc = _get_nc()
    in_maps = _in_maps(x, W, params)
    last = None
    for attempt in range(4):
        try:
            return bass_utils.run_bass_kernel_spmd(
                nc, in_maps, list(range(_NCORES)), **kw
            )
        except Exception as e:  # transient device/relay errors
            last = e
            time.sleep(2.0 * (attempt + 1))
    raise last


def kernel(x, W, params):
    res = run_spmd(x, W, params)
    return np.concatenate([res.results[c]["o"] for c in range(_NCORES)], axis=0)


# revision 8
# speedup vs baseline: 1.0957x; 1.0957x over previous
"""Trainium2 Bass kernel for nn_GateCircuit (14-qubit batched gate circuit).

Math: the reference applies RX(x@W.T[:,i]) then RY(params[i]) on wire i of
|0...0> (a product state stays a product state since each gate hits a distinct
wire), then a CNOT ladder CNOT(i, i+1), then measures <Z_0>.  Qubit 0 is only
ever a CNOT *control*, so its marginal is untouched by the ladder; the
expectation collapses to the single-qubit value

    <Z_0> = cos(x @ W[0]) * cos(params[0])
    out   = sigmoid(<Z_0>)

Sharding: pure data parallel, batch 4096 split 512 per core across 8 cores.

Host prep (scalar/row transforms only; the 4MB matvec stays on-device):
  w' = W[0] / (2*pi), and a per-call least-squares fit of
  F(v) = sigmoid(c0*cos(2*pi*sqrt(v))) on v in [0, 0.25] as a degree-4
  polynomial (c0 = cos(params[0]) known at call time; worst-case fit error
  1.8e-3 abs vs the 2e-2 rel tolerance).  w' + 5 coefficients, broadcast
  host-side to one [128, 261] tensor -> one DMA (stride-0 broadcast DMA and
  the GpSimd software-DGE queue both measured ~35GB/s -- avoided).

On-device per core (all f32, DVE only -- no ACT tables, no table loads):
  z' = x @ w'             4x DVE scalar_tensor_tensor with accumulator
                          (z' = z/2pi; x laid out [128, 4, 256]: partition p
                          holds rows 4p..4p+3; quarter-DMAs spread over the
                          Scalar/Sync hardware-DGE queues, dots ordered by
                          landing time: Scalar=[w,q0], Sync=[q1,q2,q3])
  k  = (z' + M) - M       M = 1.5*2^23: fp32 round-to-nearest
  v  = (k - z')^2         = y^2, y = frac distance, |y| <= 0.5 (sign-free)
  out= F(v)               Horner via TS/STT alternation, coeffs as
                          per-partition [128,1] APs
"""

import math

import numpy as np

_NCORES = 8
_B = 4096
_F = 256
_BS = _B // _NCORES  # 512 samples per core
_NT = _BS // 128     # 4 samples per partition
_INV_TWO_PI = float(1.0 / (2.0 * math.pi))
_MAGIC = float(1.5 * 2 ** 23)  # fp32 round-to-nearest-integer bias
_DEG = 4             # F(v) polynomial degree
_NCF = _DEG + 1      # number of coefficients

_CACHE: dict = {}


def _build():
    import concourse.bacc as bacc
    import concourse.mybir as mybir
    import concourse.tile as tile

    f32 = mybir.dt.float32
    Alu = mybir.AluOpType

    nc = bacc.Bacc("TRN2", target_bir_lowering=False, debug=False,
                   num_devices=_NCORES)

    x_d = nc.dram_tensor("x", [_BS, _F], f32, kind="ExternalInput")
    w_d = nc.dram_tensor("w", [128, _F + _NCF], f32, kind="ExternalInput")
    o_d = nc.dram_tensor("o", [_BS], f32, kind="ExternalOutput")

    with tile.TileContext(nc) as tc:
        with (
            tc.tile_pool(name="xin", bufs=1) as xpool,
            tc.tile_pool(name="scratch", bufs=2) as spool,
            tc.tile_pool(name="small", bufs=1) as zpool,
        ):
            # x quarter n holds rows 4p+n (1KB contiguous per partition).
            # Queues: Scalar HW = [w broadcast, q0], Sync HW = [q1, q2],
            # GpSimd SW = [q3]; dots run in landing order q1,q0,q2,q3.
            xt = xpool.tile([128, _NT * _F], f32)
            xr = x_d.ap().rearrange("(p n) f -> p (n f)", n=_NT)
            wz = zpool.tile([128, _F + _NCF], f32)
            nc.scalar.dma_start(wz[:], w_d[:, :])
            nc.sync.dma_start(xt[:, _F:2 * _F], xr[:, _F:2 * _F])
            nc.scalar.dma_start(xt[:, 0:_F], xr[:, 0:_F])
            nc.sync.dma_start(xt[:, 2 * _F:3 * _F], xr[:, 2 * _F:3 * _F])
            nc.sync.dma_start(xt[:, 3 * _F:], xr[:, 3 * _F:])

            # z'[p, n] = sum_f x[4p+n, f] * w'[f]
            z = zpool.tile([128, _NT], f32)
            for n in (1, 0, 2, 3):
                prod = spool.tile([128, _F], f32, name=f"prod{n}")
                nc.vector.scalar_tensor_tensor(
                    prod[:], xt[:, n * _F:(n + 1) * _F], 1.0, wz[:, 0:_F],
                    op0=Alu.mult, op1=Alu.mult,
                    accum_out=z[:, n:n + 1],
                )

            # v = (round(z') - z')^2 via the fp32 magic-number trick
            a1 = zpool.tile([128, _NT], f32)
            nc.vector.tensor_scalar(a1[:], z[:], 1.0, _MAGIC,
                                    op0=Alu.mult, op1=Alu.add)
            ny = zpool.tile([128, _NT], f32)
            nc.vector.scalar_tensor_tensor(ny[:], a1[:], -_MAGIC, z[:],
                                           op0=Alu.add, op1=Alu.subtract)
            v = zpool.tile([128, _NT], f32)
            nc.vector.tensor_tensor(v[:], ny[:], ny[:], op=Alu.mult)

            # out = f0 + v*(f1 + v*(f2 + v*(f3 + v*f4)))
            def cf(k):
                return wz[:, _F + k:_F + k + 1]

            t1 = zpool.tile([128, _NT], f32)
            nc.vector.tensor_scalar(t1[:], v[:], cf(4), cf(3),
                                    op0=Alu.mult, op1=Alu.add)
            t2 = zpool.tile([128, _NT], f32)
            nc.vector.scalar_tensor_tensor(t2[:], t1[:], 0.0, v[:],
                                           op0=Alu.bypass, op1=Alu.mult)
            t3 = zpool.tile([128, _NT], f32)
            nc.vector.scalar_tensor_tensor(t3[:], t2[:], cf(2), v[:],
                                           op0=Alu.add, op1=Alu.mult)
            t4 = zpool.tile([128, _NT], f32)
            nc.vector.scalar_tensor_tensor(t4[:], t3[:], cf(1), v[:],
                                           op0=Alu.add, op1=Alu.mult)
            ot = zpool.tile([128, _NT], f32)
            nc.vector.tensor_scalar(ot[:], t4[:], 1.0, cf(0),
                                    op0=Alu.mult, op1=Alu.add)

            nc.sync.dma_start(o_d.ap().rearrange("(p n) -> p n", n=_NT), ot[:])

    nc.compile()
    return nc


def _get_nc():
    if "nc" not in _CACHE:
        _CACHE["nc"] = _build()
    return _CACHE["nc"]


def _fit_coeffs(c0: float) -> np.ndarray:
    """Least-squares fit of sigmoid(c0*cos(2*pi*sqrt(v))) on v in [0,.25],
    degree _DEG, on Chebyshev-spaced nodes (near-minimax)."""
    t = 0.5 * (1.0 - np.cos(np.pi * np.linspace(0.0, 1.0, 401))) * 0.25
    F = 1.0 / (1.0 + np.exp(-c0 * np.cos(2.0 * np.pi * np.sqrt(t))))
    A = np.stack([t ** k for k in range(_NCF)], axis=1)
    coef, *_ = np.linalg.lstsq(A, F, rcond=None)
    return coef.astype(np.float32)


def _in_maps(x, W, params):
    x = np.ascontiguousarray(np.asarray(x, dtype=np.float32))
    W = np.asarray(W, dtype=np.float32)
    params = np.asarray(params, dtype=np.float32)
    wc = np.empty((128, _F + _NCF), dtype=np.float32)
    wc[:, :_F] = W[0] * _INV_TWO_PI
    wc[:, _F:] = _fit_coeffs(math.cos(float(params[0])))
    return [
        {"x": x[c * _BS:(c + 1) * _BS], "w": wc}
        for c in range(_NCORES)
    ]


def run_spmd(x, W, params, **kw):
    """Compile (cached) and run on 8 cores; returns BassKernelResults.

    Retries a few times: the axon-relayed device occasionally reports a
    transient NRT_EXEC_UNIT_UNRECOVERABLE that clears on the next attempt.
    """
    import time

    from concourse import bass_utils

    nc = _get_nc()
    in_maps = _in_maps(x, W, params)
    last = None
    for attempt in range(4):
        try:
            return bass_utils.run_bass_kernel_spmd(
                nc, in_maps, list(range(_NCORES)), **kw
            )
        except Exception as e:  # transient device/relay errors
            last = e
            time.sleep(2.0 * (attempt + 1))
    raise last


def kernel(x, W, params):
    res = run_spmd(x, W, params)
    return np.concatenate([res.results[c]["o"] for c in range(_NCORES)], axis=0)


# revision 9
# speedup vs baseline: 1.1352x; 1.0361x over previous
"""Trainium2 Bass kernel for nn_GateCircuit (14-qubit batched gate circuit).

Math: the reference applies RX(x@W.T[:,i]) then RY(params[i]) on wire i of
|0...0> (a product state stays a product state since each gate hits a distinct
wire), then a CNOT ladder CNOT(i, i+1), then measures <Z_0>.  Qubit 0 is only
ever a CNOT *control*, so its marginal is untouched by the ladder; the
expectation collapses to the single-qubit value

    <Z_0> = cos(x @ W[0]) * cos(params[0])
    out   = sigmoid(<Z_0>)

Sharding: pure data parallel, batch 4096 split 512 per core across 8 cores.

Host prep (scalar/row transforms only; the 4MB matvec stays on-device):
  w' = W[0] / (2*pi) as a single [1, 256] row (1KB DMA; the two hardware
  DMA queues run ~107GB/s each, so keeping the 131KB host-side broadcast
  off them is worth ~0.6us).  A per-call least-squares fit of
  F(v) = sigmoid(c0*cos(2*pi*sqrt(v))) on v in [0, 0.25], degree 4
  (worst-case fit error 1.8e-3 abs vs the 2e-2 rel tolerance), is baked
  into the NEFF as immediates -- the compile cache is keyed on the
  coefficient values, so a params change recompiles (correctness first).

On-device per core (all f32, DVE + one PE op -- no ACT tables):
  wP  = ones[128,1] @ w'[1,256]   PE rank-1 broadcast into PSUM
  z'  = x @ w'                    4x DVE scalar_tensor_tensor + accumulator
                                  (z' = z/2pi; x laid out [128, 4, 256]:
                                  partition p holds rows 4p..4p+3; quarter
                                  DMAs balanced over the Scalar/Sync HW-DGE
                                  queues, dots ordered by landing time)
  k   = (z' + M) - M              M = 1.5*2^23: fp32 round-to-nearest
  v   = (k - z')^2                = y^2, |y| <= 0.5 (sign-free)
  out = F(v)                      Horner via TS/STT, immediate coeffs
"""

import math

import numpy as np

_NCORES = 8
_B = 4096
_F = 256
_BS = _B // _NCORES  # 512 samples per core
_NT = _BS // 128     # 4 samples per partition
_INV_TWO_PI = float(1.0 / (2.0 * math.pi))
_MAGIC = float(1.5 * 2 ** 23)  # fp32 round-to-nearest-integer bias
_DEG = 4             # F(v) polynomial degree
_NCF = _DEG + 1      # number of coefficients

_CACHE: dict = {}


def _build(coeffs):
    import concourse.bacc as bacc
    import concourse.mybir as mybir
    import concourse.tile as tile

    f32 = mybir.dt.float32
    Alu = mybir.AluOpType
    f0, f1, f2, f3, f4 = (float(c) for c in coeffs)

    nc = bacc.Bacc("TRN2", target_bir_lowering=False, debug=False,
                   num_devices=_NCORES)

    x_d = nc.dram_tensor("x", [_BS, _F], f32, kind="ExternalInput")
    w_d = nc.dram_tensor("w", [1, _F], f32, kind="ExternalInput")
    o_d = nc.dram_tensor("o", [_BS], f32, kind="ExternalOutput")

    with tile.TileContext(nc) as tc:
        with (
            tc.tile_pool(name="xin", bufs=1) as xpool,
            tc.tile_pool(name="scratch", bufs=2) as spool,
            tc.tile_pool(name="small", bufs=1) as zpool,
            tc.tile_pool(name="wps", bufs=1, space="PSUM") as ppool,
        ):
            # x quarter n holds rows 4p+n (1KB contiguous per partition).
            # Queues: Scalar HW = [w row, q0, q1], Sync HW = [q2, q3];
            # dots run in landing order q2, q0, q3, q1.
            xt = xpool.tile([128, _NT * _F], f32)
            xr = x_d.ap().rearrange("(p n) f -> p (n f)", n=_NT)
            wrow = zpool.tile([1, _F], f32)
            nc.scalar.dma_start(wrow[:], w_d[:, :])
            nc.sync.dma_start(xt[:, 2 * _F:3 * _F], xr[:, 2 * _F:3 * _F])
            nc.scalar.dma_start(xt[:, 0:_F], xr[:, 0:_F])
            nc.sync.dma_start(xt[:, 3 * _F:], xr[:, 3 * _F:])
            nc.scalar.dma_start(xt[:, _F:2 * _F], xr[:, _F:2 * _F])

            # broadcast w' to all partitions: wP = ones[128,1] @ wrow
            ones = zpool.tile([1, 128], f32)
            nc.gpsimd.memset(ones[:], 1.0)
            wP = ppool.tile([128, _F], f32)
            nc.tensor.matmul(wP[:], ones[:], wrow[:], start=True, stop=True)

            # z'[p, n] = sum_f x[4p+n, f] * w'[f]
            z = zpool.tile([128, _NT], f32)
            for n in (2, 0, 3, 1):
                prod = spool.tile([128, _F], f32, name=f"prod{n}")
                nc.vector.scalar_tensor_tensor(
                    prod[:], xt[:, n * _F:(n + 1) * _F], 1.0, wP[:],
                    op0=Alu.mult, op1=Alu.mult,
                    accum_out=z[:, n:n + 1],
                )

            # v = (round(z') - z')^2 via the fp32 magic-number trick
            a1 = zpool.tile([128, _NT], f32)
            nc.vector.tensor_scalar(a1[:], z[:], 1.0, _MAGIC,
                                    op0=Alu.mult, op1=Alu.add)
            ny = zpool.tile([128, _NT], f32)
            nc.vector.scalar_tensor_tensor(ny[:], a1[:], -_MAGIC, z[:],
                                           op0=Alu.add, op1=Alu.subtract)
            v = zpool.tile([128, _NT], f32)
            nc.vector.tensor_tensor(v[:], ny[:], ny[:], op=Alu.mult)

            # out = f0 + v*(f1 + v*(f2 + v*(f3 + v*f4))), immediates
            t1 = zpool.tile([128, _NT], f32)
            nc.vector.tensor_scalar(t1[:], v[:], f4, f3,
                                    op0=Alu.mult, op1=Alu.add)
            t2 = zpool.tile([128, _NT], f32)
            nc.vector.scalar_tensor_tensor(t2[:], t1[:], 0.0, v[:],
                                           op0=Alu.bypass, op1=Alu.mult)
            t3 = zpool.tile([128, _NT], f32)
            nc.vector.scalar_tensor_tensor(t3[:], t2[:], f2, v[:],
                                           op0=Alu.add, op1=Alu.mult)
            t4 = zpool.tile([128, _NT], f32)
            nc.vector.scalar_tensor_tensor(t4[:], t3[:], f1, v[:],
                                           op0=Alu.add, op1=Alu.mult)
            ot = zpool.tile([128, _NT], f32)
            nc.vector.tensor_scalar(ot[:], t4[:], 1.0, f0,
                                    op0=Alu.mult, op1=Alu.add)

            nc.sync.dma_start(o_d.ap().rearrange("(p n) -> p n", n=_NT), ot[:])

    nc.compile()
    return nc


def _get_nc(coeffs):
    key = tuple(float(c) for c in coeffs)
    if _CACHE.get("key") != key:
        _CACHE["nc"] = _build(coeffs)
        _CACHE["key"] = key
    return _CACHE["nc"]


def _fit_coeffs(c0: float) -> np.ndarray:
    """Least-squares fit of sigmoid(c0*cos(2*pi*sqrt(v))) on v in [0,.25],
    degree _DEG, on Chebyshev-spaced nodes (near-minimax)."""
    t = 0.5 * (1.0 - np.cos(np.pi * np.linspace(0.0, 1.0, 401))) * 0.25
    F = 1.0 / (1.0 + np.exp(-c0 * np.cos(2.0 * np.pi * np.sqrt(t))))
    A = np.stack([t ** k for k in range(_NCF)], axis=1)
    coef, *_ = np.linalg.lstsq(A, F, rcond=None)
    return coef.astype(np.float32)


def _in_maps(x, W):
    x = np.ascontiguousarray(np.asarray(x, dtype=np.float32))
    W = np.asarray(W, dtype=np.float32)
    wc = np.ascontiguousarray((W[0] * _INV_TWO_PI).reshape(1, _F))
    return [
        {"x": x[c * _BS:(c + 1) * _BS], "w": wc}
        for c in range(_NCORES)
    ]


def run_spmd(x, W, params, **kw):
    """Compile (cached per params) and run on 8 cores.

    Retries a few times: the axon-relayed device occasionally reports a
    transient NRT_EXEC_UNIT_UNRECOVERABLE that clears on the next attempt.
    """
    import time

    from concourse import bass_utils

    params = np.asarray(params, dtype=np.float32)
    coeffs = _fit_coeffs(math.cos(float(params[0])))
    nc = _get_nc(coeffs)
    in_maps = _in_maps(x, W)
    last = None
    for attempt in range(4):
        try:
            return bass_utils.run_bass_kernel_spmd(
                nc, in_maps, list(range(_NCORES)), **kw
            )
        except Exception as e:  # transient device/relay errors
            last = e
            time.sleep(2.0 * (attempt + 1))
    raise last


def kernel(x, W, params):
    res = run_spmd(x, W, params)
    return np.concatenate([res.results[c]["o"] for c in range(_NCORES)], axis=0)
